# revision 9
# baseline (speedup 1.0000x reference)
"""GQA attention kernel for Trainium2: B=1, S=4096, D=1024, H=8 heads (hd=128).

Sharding: one head per NeuronCore (8 cores). Each core computes its head's
Q/K/V projections from the full hidden states, then causal flash-style
attention on-chip, writing its context slice as ctx^T [hd, S] (fp16, host
transposes + upcasts).

Per-core design (fp16 matmul operands, fp32 PSUM):
  - projections and attention are INTERLEAVED: after projecting chunks
    2p,2p+1 of S, the attention for q-chunk pair (2p, 2p+1) is emitted (it
    only needs K/V blocks up to chunk 2p+1), so ACT exp / DVE accum backlogs
    drain under the next chunks' projection matmuls and the PE never idles
  - V^T -> V-natural via DMA XBAR transpose (off the PE)
  - score pairs land in [128,1024] PSUM tiles so exp is one wide ACT op
  - diagonal tiles (both chunks') are emitted FIRST within a pair so their
    serial exp->mask(GPSIMD)->accum chains hide under later full tiles
  - causal diagonal tiles compute only the live slice; mask = one shared
    [128,128] upper-tri multiply on the triangle
  - softmax denominator: DVE accumulates exp tiles into 2 fp16 accumulators
    per q-chunk; TWO accumulating ones-matmuls replicate Z across partitions
    (no DVE merge); reciprocal_approx_fast
  - PV runs in emission order (sum order is free), trailing by 2 tiles
"""

import os
from contextlib import ExitStack

import numpy as np

B, S, D = 1, 4096, 1024
H = 8
HD = D // H  # 128
P = 128
QC = 512  # q-chunk (columns per scores tile)
NDC = D // P  # 8 d-chunks
NQC = S // QC  # 8 q-chunks
NKB = S // P  # 32 k-blocks
SCALE = 1.0 / float(np.sqrt(HD))
LAG = 2  # PV trails score-tile emission by this many tiles
CHUNK = NDC * QC


def _build_program():
    nc = _build_program_inner()
    nc.finalize()
    return nc


def _build_program_inner():
    from concourse import bacc, mybir, tile

    f32 = mybir.dt.float32
    f16 = mybir.dt.float16

    nc = bacc.Bacc("TRN2", target_bir_lowering=False, debug=False)

    # xh[p, n*4096 + d*512 + c] = x[512n + c, 128d + p]
    xh = nc.dram_tensor("xh", [P, S * NDC], f16, kind="ExternalInput")
    # w*[p, d*128 + o] = W[128h + o, 128d + p] for this core's head h
    wq = nc.dram_tensor("wq", [P, D], f16, kind="ExternalInput")
    wk = nc.dram_tensor("wk", [P, D], f16, kind="ExternalInput")
    wv = nc.dram_tensor("wv", [P, D], f16, kind="ExternalInput")
    # tri[r, c] = 1.0 if c >= r else 0.0 (upper triangular incl. diagonal)
    tri = nc.dram_tensor("tri", [P, P], f16, kind="ExternalInput")
    out = nc.dram_tensor("out", [HD, S], f16, kind="ExternalOutput")

    Exp = mybir.ActivationFunctionType.Exp

    with ExitStack() as stack:
        tc = stack.enter_context(tile.TileContext(nc))
        constp = stack.enter_context(tc.tile_pool(name="const", bufs=1))
        qkvp = stack.enter_context(tc.tile_pool(name="qkv", bufs=1))
        xp = stack.enter_context(tc.tile_pool(name="x", bufs=1))
        wp = stack.enter_context(tc.tile_pool(name="w", bufs=1))
        vtp = stack.enter_context(tc.tile_pool(name="vt", bufs=1))
        expp = stack.enter_context(tc.tile_pool(name="expp", bufs=10))
        accp = stack.enter_context(tc.tile_pool(name="accp", bufs=8))
        finp = stack.enter_context(tc.tile_pool(name="fin", bufs=2))
        csp = stack.enter_context(tc.tile_pool(name="csp", bufs=2))
        pp1 = stack.enter_context(tc.tile_pool(name="pp1", bufs=2, space="PSUM"))
        ps_s = stack.enter_context(tc.tile_pool(name="ps_s", bufs=2, space="PSUM"))
        ps_c = stack.enter_context(tc.tile_pool(name="ps_c", bufs=2, space="PSUM"))

        ones_sq = constp.tile([P, P], f16, tag="ones_sq")
        nc.gpsimd.memset(ones_sq[:], 1.0)
        tri_sb = constp.tile([P, P], f16, tag="tri")

        qt_sb = qkvp.tile([P, S], f16, tag="qt")
        kt_sb = qkvp.tile([P, S], f16, tag="kt")
        vn_sb = qkvp.tile([P, S], f16, tag="vn")  # V natural: 32 blocks [128k,128hd]
        xsb = xp.tile([P, S * NDC], f16, tag="xsb")
        vt_sb = vtp.tile([P, S], f16, tag="vt")

        w_sb = {}
        for name in ("q", "k", "v"):
            w_sb[name] = wp.tile([P, D], f16, tag=f"w{name}", name=f"w{name}")
        # DMA order: first matmul needs only wk + x chunk 0
        nc.sync.dma_start(out=w_sb["k"][:], in_=wk[:, :])
        nc.scalar.dma_start(out=xsb[:, 0:CHUNK], in_=xh[:, 0:CHUNK])
        nc.sync.dma_start(out=w_sb["q"][:], in_=wq[:, :])
        nc.scalar.dma_start(out=xsb[:, CHUNK:2 * CHUNK], in_=xh[:, CHUNK:2 * CHUNK])
        nc.sync.dma_start(out=w_sb["v"][:], in_=wv[:, :])
        nc.sync.dma_start(out=tri_sb[:], in_=tri[:, :])
        for n in range(2, NQC):
            eng = nc.sync if n % 2 == 0 else nc.scalar
            eng.dma_start(
                out=xsb[:, n * CHUNK:(n + 1) * CHUNK],
                in_=xh[:, n * CHUNK:(n + 1) * CHUNK],
            )

        def proj_chunk(n):
            xbase = n * CHUNK
            for name, dst in (("k", kt_sb), ("q", qt_sb), ("v", vt_sb)):
                ps = pp1.tile([P, QC], f32, tag="qkvps", name=f"ps{n}{name}")
                for d in range(NDC):
                    nc.tensor.matmul(
                        out=ps[:],
                        lhsT=w_sb[name][:, d * HD:(d + 1) * HD],
                        rhs=xsb[:, xbase + d * QC:xbase + (d + 1) * QC],
                        start=(d == 0),
                        stop=(d == NDC - 1),
                    )
                nc.vector.tensor_copy(out=dst[:, n * QC:(n + 1) * QC], in_=ps[:])

        def pair_attention(pr):
            qA, qB = 2 * pr, 2 * pr + 1
            nkA, nkB = 4 * qA + 4, 4 * qB + 4
            baseA, baseB = qA * QC, qB * QC

            accA = [
                accp.tile([P, QC], f16, tag="acc", name=f"accA{pr}{i}")
                for i in range(2)
            ]
            accB = [
                accp.tile([P, QC], f16, tag="acc", name=f"accB{pr}{i}")
                for i in range(2)
            ]
            c_psA = ps_c.tile([P, QC], f32, tag="cps", name=f"cA{pr}")
            c_psB = ps_c.tile([P, QC], f32, tag="cps", name=f"cB{pr}")

            # emission order: B diagonals, A diagonals (as pair tiles), then
            # full-full pair tiles in k order. PV consumes in the same order.
            order = (
                list(range(nkA, nkB))          # B-only diagonal tiles
                + list(range(4 * qA, nkA))     # A-diagonal pair tiles
                + list(range(4 * qA))          # full-full pair tiles
            )
            a_tiles = [ki for ki in order if ki < nkA]
            firstA, lastA = a_tiles[0], order[-1]
            firstB, lastB = order[0], order[-1]
            # per-qc accumulator bookkeeping: emission counts per chunk
            na_seen = [0]
            nb_seen = [0]
            exps = {}  # ki -> (e_tile, loA or None, loB)

            def acc_update(acc, seen, lo, e_ap):
                i = seen[0]
                seen[0] += 1
                par = i % 2
                if i == 0:
                    nc.vector.tensor_copy(out=acc[0][:], in_=e_ap)
                elif i == 1:
                    if lo > 0:
                        nc.vector.memset(acc[1][:, :lo], 0.0)
                    nc.vector.tensor_copy(out=acc[1][:, lo:], in_=e_ap)
                else:
                    nc.vector.tensor_add(
                        out=acc[par][:, lo:], in0=acc[par][:, lo:], in1=e_ap
                    )

            def pv(k):
                e, loA, loB = exps[k]
                if loA is not None:
                    nc.tensor.matmul(
                        out=c_psA[:, loA:],
                        lhsT=vn_sb[:, k * P:(k + 1) * P],
                        rhs=e[:, loA:QC],
                        start=(k == firstA),
                        stop=(k == lastA),
                    )
                nc.tensor.matmul(
                    out=c_psB[:, loB:],
                    lhsT=vn_sb[:, k * P:(k + 1) * P],
                    rhs=e[:, QC + loB:2 * QC] if loA is not None
                    else e[:, loB:QC],
                    start=(k == firstB),
                    stop=(k == lastB),
                )

            def qc_tail(q0, acc, c_ps, base):
                z_ps = ps_s.tile([P, 2 * QC], f32, tag="sps", name=f"z{q0}")
                nc.tensor.matmul(
                    out=z_ps[:, :QC], lhsT=ones_sq[:], rhs=acc[0][:],
                    start=True, stop=False,
                )
                nc.tensor.matmul(
                    out=z_ps[:, :QC], lhsT=ones_sq[:], rhs=acc[1][:],
                    start=False, stop=True,
                )
                rz = finp.tile([P, QC], f32, tag="rz", name=f"rz{q0}")
                nc.vector.reciprocal_approx_fast(out=rz[:], in_=z_ps[:, :QC])
                cs = csp.tile([P, QC], f16, tag="cs", name=f"cs{q0}")
                nc.vector.tensor_mul(out=cs[:], in0=c_ps[:], in1=rz[:])
                nc.sync.dma_start(out=out[:, base:base + QC], in_=cs[:])

            for idx, ki in enumerate(order):
                loB = P * (ki - 4 * qB) if ki >= 4 * qB else 0
                if ki < nkA:  # pair tile (A live, B full)
                    loA = P * (ki - 4 * qA) if ki >= 4 * qA else 0
                    s = ps_s.tile([P, 2 * QC], f32, tag="sps", name=f"s{pr}_{ki}")
                    nc.tensor.matmul(
                        out=s[:, loA:QC],
                        lhsT=kt_sb[:, ki * P:(ki + 1) * P],
                        rhs=qt_sb[:, baseA + loA:baseA + QC],
                        start=True, stop=True,
                    )
                    nc.tensor.matmul(
                        out=s[:, QC:],
                        lhsT=kt_sb[:, ki * P:(ki + 1) * P],
                        rhs=qt_sb[:, baseB:baseB + QC],
                        start=True, stop=True,
                    )
                    e = expp.tile([P, 2 * QC], f16, tag="exp", name=f"e{pr}_{ki}")
                    nc.scalar.activation(
                        out=e[:, loA:], in_=s[:, loA:], func=Exp, scale=SCALE
                    )
                    if ki >= 4 * qA:  # diagonal for A: mask the triangle
                        nc.gpsimd.tensor_mul(
                            out=e[:, loA:loA + P], in0=e[:, loA:loA + P],
                            in1=tri_sb[:],
                        )
                    acc_update(accA, na_seen, loA, e[:, loA:QC])
                    acc_update(accB, nb_seen, 0, e[:, QC:])
                    exps[ki] = (e, loA, 0)
                else:  # B-only diagonal tile, emitted first
                    s = ps_s.tile([P, 2 * QC], f32, tag="sps", name=f"s{pr}_{ki}")
                    nc.tensor.matmul(
                        out=s[:, loB:QC],
                        lhsT=kt_sb[:, ki * P:(ki + 1) * P],
                        rhs=qt_sb[:, baseB + loB:baseB + QC],
                        start=True, stop=True,
                    )
                    e = expp.tile([P, 2 * QC], f16, tag="exp", name=f"e{pr}_{ki}")
                    nc.scalar.activation(
                        out=e[:, loB:QC], in_=s[:, loB:QC], func=Exp, scale=SCALE
                    )
                    nc.gpsimd.tensor_mul(
                        out=e[:, loB:loB + P], in0=e[:, loB:loB + P],
                        in1=tri_sb[:],
                    )
                    acc_update(accB, nb_seen, loB, e[:, loB:QC])
                    exps[ki] = (e, None, loB)
                # PV trails emission by LAG tiles, in emission order
                if idx - LAG >= 0:
                    pv(order[idx - LAG])
            for k in order[len(order) - LAG:]:
                pv(k)
            qc_tail(qA, accA, c_psA, baseA)
            qc_tail(qB, accB, c_psB, baseB)

        # ------------- interleaved projection + attention schedule -------------
        for pr in range(NQC // 2):
            proj_chunk(2 * pr)
            proj_chunk(2 * pr + 1)
            # V^T chunks 2pr,2pr+1 -> V natural blocks via DMA XBAR transpose
            lo, hi = 2 * pr * QC, (2 * pr + 2) * QC
            nc.sync.dma_start_transpose(
                out=vn_sb[:, lo:hi].rearrange("p (b c) -> p b c", c=P),
                in_=vt_sb[:, lo:hi],
            )
            pair_attention(pr)

    return nc


_NC_CACHE = None


def _get_nc():
    global _NC_CACHE
    if _NC_CACHE is None:
        _NC_CACHE = _build_program()
    return _NC_CACHE


def _prep_inputs(hidden_states, Wq, Wk, Wv):
    x = np.asarray(hidden_states, dtype=np.float32).reshape(S, D)
    xh = np.ascontiguousarray(
        x.reshape(NQC, QC, NDC, P).transpose(3, 0, 2, 1).reshape(P, S * NDC)
    ).astype(np.float16)
    tri = np.triu(np.ones((P, P), dtype=np.float16))

    def wprep(W, h):
        Wh = np.asarray(W, dtype=np.float32)[h * HD:(h + 1) * HD, :]  # [o, in]
        return np.ascontiguousarray(
            Wh.reshape(HD, NDC, P).transpose(2, 1, 0).reshape(P, D)
        ).astype(np.float16)

    in_maps = []
    for h in range(H):
        in_maps.append({
            "xh": xh,
            "wq": wprep(Wq, h),
            "wk": wprep(Wk, h),
            "wv": wprep(Wv, h),
            "tri": tri,
        })
    return in_maps


def kernel(hidden_states, Wq, Wk, Wv, trace=False, **trace_kwargs):
    from concourse.bass_utils import run_bass_kernel_spmd

    in_maps = _prep_inputs(hidden_states, Wq, Wk, Wv)
    nc = _get_nc()
    res = run_bass_kernel_spmd(
        nc, in_maps, core_ids=list(range(H)), trace=trace, **trace_kwargs
    )
    ctx = np.empty((B, S, D), dtype=np.float32)
    for h in range(H):
        ctx[0, :, h * HD:(h + 1) * HD] = res.results[h]["out"].T.astype(np.float32)
    if trace:
        return ctx, res
    return ctx


# revision 10
# speedup vs baseline: 1.0384x; 1.0384x over previous
"""GQA attention kernel for Trainium2: B=1, S=4096, D=1024, H=8 heads (hd=128).

Sharding: one head per NeuronCore (8 cores). Each core computes its head's
Q/K/V projections from the full hidden states, then causal flash-style
attention on-chip, writing its context slice as ctx^T [hd, S] (fp16, host
transposes + upcasts).

Per-core design (fp16 matmul operands, fp32 PSUM):
  - projections and attention are INTERLEAVED: after projecting chunks
    2p,2p+1 of S, the attention for q-chunk pair (2p, 2p+1) is emitted (it
    only needs K/V blocks up to chunk 2p+1), so ACT exp / DVE accum backlogs
    drain under the next chunks' projection matmuls and the PE never idles
  - V^T -> V-natural via DMA XBAR transpose (off the PE)
  - score pairs land in [128,1024] PSUM tiles so exp is one wide ACT op
  - diagonal tiles (both chunks') are emitted FIRST within a pair so their
    serial exp->mask(GPSIMD)->accum chains hide under later full tiles
  - causal diagonal tiles compute only the live slice; mask = one shared
    [128,128] upper-tri multiply on the triangle
  - softmax denominator: DVE accumulates exp tiles into 2 fp16 accumulators
    per q-chunk; TWO accumulating ones-matmuls replicate Z across partitions
    (no DVE merge); reciprocal_approx_fast
  - PV runs in emission order (sum order is free), trailing by 2 tiles
"""

import os
from contextlib import ExitStack

import numpy as np

B, S, D = 1, 4096, 1024
H = 8
HD = D // H  # 128
P = 128
QC = 512  # q-chunk (columns per scores tile)
NDC = D // P  # 8 d-chunks
NQC = S // QC  # 8 q-chunks
NKB = S // P  # 32 k-blocks
SCALE = 1.0 / float(np.sqrt(HD))
LAG = 2  # PV trails score-tile emission by this many tiles
CHUNK = NDC * QC


def _build_program():
    nc = _build_program_inner()
    nc.finalize()
    return nc


def _build_program_inner():
    from concourse import bacc, mybir, tile

    f32 = mybir.dt.float32
    f16 = mybir.dt.float16

    nc = bacc.Bacc("TRN2", target_bir_lowering=False, debug=False)

    # xh[p, n*4096 + d*512 + c] = x[512n + c, 128d + p]
    xh = nc.dram_tensor("xh", [P, S * NDC], f16, kind="ExternalInput")
    # w*[p, d*128 + o] = W[128h + o, 128d + p] for this core's head h
    wq = nc.dram_tensor("wq", [P, D], f16, kind="ExternalInput")
    wk = nc.dram_tensor("wk", [P, D], f16, kind="ExternalInput")
    wv = nc.dram_tensor("wv", [P, D], f16, kind="ExternalInput")
    # tri[r, c] = 1.0 if c >= r else 0.0 (upper triangular incl. diagonal)
    tri = nc.dram_tensor("tri", [P, P], f16, kind="ExternalInput")
    out = nc.dram_tensor("out", [HD, S], f16, kind="ExternalOutput")

    Exp = mybir.ActivationFunctionType.Exp

    with ExitStack() as stack:
        tc = stack.enter_context(tile.TileContext(nc))
        constp = stack.enter_context(tc.tile_pool(name="const", bufs=1))
        qkvp = stack.enter_context(tc.tile_pool(name="qkv", bufs=1))
        xp = stack.enter_context(tc.tile_pool(name="x", bufs=1))
        wp = stack.enter_context(tc.tile_pool(name="w", bufs=1))
        vtp = stack.enter_context(tc.tile_pool(name="vt", bufs=1))
        expp = stack.enter_context(tc.tile_pool(name="expp", bufs=10))
        accp = stack.enter_context(tc.tile_pool(name="accp", bufs=8))
        finp = stack.enter_context(tc.tile_pool(name="fin", bufs=2))
        csp = stack.enter_context(tc.tile_pool(name="csp", bufs=2))
        pp1 = stack.enter_context(tc.tile_pool(name="pp1", bufs=2, space="PSUM"))
        ps_s = stack.enter_context(tc.tile_pool(name="ps_s", bufs=2, space="PSUM"))
        ps_c = stack.enter_context(tc.tile_pool(name="ps_c", bufs=2, space="PSUM"))

        ones_sq = constp.tile([P, P], f16, tag="ones_sq")
        nc.gpsimd.memset(ones_sq[:], 1.0)
        tri_sb = constp.tile([P, P], f16, tag="tri")

        qt_sb = qkvp.tile([P, S], f16, tag="qt")
        kt_sb = qkvp.tile([P, S], f16, tag="kt")
        vn_sb = qkvp.tile([P, S], f16, tag="vn")  # V natural: 32 blocks [128k,128hd]
        xsb = xp.tile([P, S * NDC], f16, tag="xsb")
        vt_sb = vtp.tile([P, S], f16, tag="vt")

        w_sb = {}
        for name in ("q", "k", "v"):
            w_sb[name] = wp.tile([P, D], f16, tag=f"w{name}", name=f"w{name}")
        # Queue split: sync carries only small latency-critical transfers
        # (weights, tri, V transposes, output); scalar carries the bulk xh
        # prefetch so it never delays them. Chunks 0-1 stream per-d-slice so
        # the first projection matmul starts after ~130KB, not 1MB.
        nc.sync.dma_start(out=w_sb["k"][:], in_=wk[:, :])
        for n in (0, 1):
            for d in range(NDC):
                lo = n * CHUNK + d * QC
                nc.scalar.dma_start(out=xsb[:, lo:lo + QC], in_=xh[:, lo:lo + QC])
        nc.sync.dma_start(out=w_sb["q"][:], in_=wq[:, :])
        nc.sync.dma_start(out=w_sb["v"][:], in_=wv[:, :])
        nc.sync.dma_start(out=tri_sb[:], in_=tri[:, :])
        for n in range(2, NQC):
            nc.scalar.dma_start(
                out=xsb[:, n * CHUNK:(n + 1) * CHUNK],
                in_=xh[:, n * CHUNK:(n + 1) * CHUNK],
            )

        def proj_chunk(n):
            xbase = n * CHUNK
            for name, dst in (("k", kt_sb), ("q", qt_sb), ("v", vt_sb)):
                ps = pp1.tile([P, QC], f32, tag="qkvps", name=f"ps{n}{name}")
                for d in range(NDC):
                    nc.tensor.matmul(
                        out=ps[:],
                        lhsT=w_sb[name][:, d * HD:(d + 1) * HD],
                        rhs=xsb[:, xbase + d * QC:xbase + (d + 1) * QC],
                        start=(d == 0),
                        stop=(d == NDC - 1),
                    )
                nc.vector.tensor_copy(out=dst[:, n * QC:(n + 1) * QC], in_=ps[:])

        def pair_attention(pr):
            qA, qB = 2 * pr, 2 * pr + 1
            nkA, nkB = 4 * qA + 4, 4 * qB + 4
            baseA, baseB = qA * QC, qB * QC

            accA = [
                accp.tile([P, QC], f16, tag="acc", name=f"accA{pr}{i}")
                for i in range(2)
            ]
            accB = [
                accp.tile([P, QC], f16, tag="acc", name=f"accB{pr}{i}")
                for i in range(2)
            ]
            c_psA = ps_c.tile([P, QC], f32, tag="cps", name=f"cA{pr}")
            c_psB = ps_c.tile([P, QC], f32, tag="cps", name=f"cB{pr}")

            # emission order: B diagonals, A diagonals (as pair tiles), then
            # full-full pair tiles in k order. PV consumes in the same order.
            order = (
                list(range(nkA, nkB))          # B-only diagonal tiles
                + list(range(4 * qA, nkA))     # A-diagonal pair tiles
                + list(range(4 * qA))          # full-full pair tiles
            )
            a_tiles = [ki for ki in order if ki < nkA]
            firstA, lastA = a_tiles[0], order[-1]
            firstB, lastB = order[0], order[-1]
            # per-qc accumulator bookkeeping: emission counts per chunk
            na_seen = [0]
            nb_seen = [0]
            exps = {}  # ki -> (e_tile, loA or None, loB)

            def acc_update(acc, seen, lo, e_ap):
                i = seen[0]
                seen[0] += 1
                par = i % 2
                if i == 0:
                    nc.vector.tensor_copy(out=acc[0][:], in_=e_ap)
                elif i == 1:
                    if lo > 0:
                        nc.vector.memset(acc[1][:, :lo], 0.0)
                    nc.vector.tensor_copy(out=acc[1][:, lo:], in_=e_ap)
                else:
                    nc.vector.tensor_add(
                        out=acc[par][:, lo:], in0=acc[par][:, lo:], in1=e_ap
                    )

            def pv(k):
                e, loA, loB = exps[k]
                if loA is not None:
                    nc.tensor.matmul(
                        out=c_psA[:, loA:],
                        lhsT=vn_sb[:, k * P:(k + 1) * P],
                        rhs=e[:, loA:QC],
                        start=(k == firstA),
                        stop=(k == lastA),
                    )
                nc.tensor.matmul(
                    out=c_psB[:, loB:],
                    lhsT=vn_sb[:, k * P:(k + 1) * P],
                    rhs=e[:, QC + loB:2 * QC] if loA is not None
                    else e[:, loB:QC],
                    start=(k == firstB),
                    stop=(k == lastB),
                )

            def qc_tail(q0, acc, c_ps, base):
                z_ps = ps_s.tile([P, 2 * QC], f32, tag="sps", name=f"z{q0}")
                nc.tensor.matmul(
                    out=z_ps[:, :QC], lhsT=ones_sq[:], rhs=acc[0][:],
                    start=True, stop=False,
                )
                nc.tensor.matmul(
                    out=z_ps[:, :QC], lhsT=ones_sq[:], rhs=acc[1][:],
                    start=False, stop=True,
                )
                rz = finp.tile([P, QC], f32, tag="rz", name=f"rz{q0}")
                nc.vector.reciprocal_approx_fast(out=rz[:], in_=z_ps[:, :QC])
                cs = csp.tile([P, QC], f16, tag="cs", name=f"cs{q0}")
                nc.vector.tensor_mul(out=cs[:], in0=c_ps[:], in1=rz[:])
                nc.sync.dma_start(out=out[:, base:base + QC], in_=cs[:])

            for idx, ki in enumerate(order):
                loB = P * (ki - 4 * qB) if ki >= 4 * qB else 0
                if ki < nkA:  # pair tile (A live, B full)
                    loA = P * (ki - 4 * qA) if ki >= 4 * qA else 0
                    s = ps_s.tile([P, 2 * QC], f32, tag="sps", name=f"s{pr}_{ki}")
                    nc.tensor.matmul(
                        out=s[:, loA:QC],
                        lhsT=kt_sb[:, ki * P:(ki + 1) * P],
                        rhs=qt_sb[:, baseA + loA:baseA + QC],
                        start=True, stop=True,
                    )
                    nc.tensor.matmul(
                        out=s[:, QC:],
                        lhsT=kt_sb[:, ki * P:(ki + 1) * P],
                        rhs=qt_sb[:, baseB:baseB + QC],
                        start=True, stop=True,
                    )
                    e = expp.tile([P, 2 * QC], f16, tag="exp", name=f"e{pr}_{ki}")
                    nc.scalar.activation(
                        out=e[:, loA:], in_=s[:, loA:], func=Exp, scale=SCALE
                    )
                    if ki >= 4 * qA:  # diagonal for A: mask the triangle
                        nc.gpsimd.tensor_mul(
                            out=e[:, loA:loA + P], in0=e[:, loA:loA + P],
                            in1=tri_sb[:],
                        )
                    acc_update(accA, na_seen, loA, e[:, loA:QC])
                    acc_update(accB, nb_seen, 0, e[:, QC:])
                    exps[ki] = (e, loA, 0)
                else:  # B-only diagonal tile, emitted first
                    s = ps_s.tile([P, 2 * QC], f32, tag="sps", name=f"s{pr}_{ki}")
                    nc.tensor.matmul(
                        out=s[:, loB:QC],
                        lhsT=kt_sb[:, ki * P:(ki + 1) * P],
                        rhs=qt_sb[:, baseB + loB:baseB + QC],
                        start=True, stop=True,
                    )
                    e = expp.tile([P, 2 * QC], f16, tag="exp", name=f"e{pr}_{ki}")
                    nc.scalar.activation(
                        out=e[:, loB:QC], in_=s[:, loB:QC], func=Exp, scale=SCALE
                    )
                    nc.gpsimd.tensor_mul(
                        out=e[:, loB:loB + P], in0=e[:, loB:loB + P],
                        in1=tri_sb[:],
                    )
                    acc_update(accB, nb_seen, loB, e[:, loB:QC])
                    exps[ki] = (e, None, loB)
                # PV trails emission by LAG tiles, in emission order
                if idx - LAG >= 0:
                    pv(order[idx - LAG])
            for k in order[len(order) - LAG:]:
                pv(k)
            qc_tail(qA, accA, c_psA, baseA)
            qc_tail(qB, accB, c_psB, baseB)

        # ------------- interleaved projection + attention schedule -------------
        for pr in range(NQC // 2):
            proj_chunk(2 * pr)
            proj_chunk(2 * pr + 1)
            # V^T chunks 2pr,2pr+1 -> V natural blocks via DMA XBAR transpose
            lo, hi = 2 * pr * QC, (2 * pr + 2) * QC
            nc.sync.dma_start_transpose(
                out=vn_sb[:, lo:hi].rearrange("p (b c) -> p b c", c=P),
                in_=vt_sb[:, lo:hi],
            )
            pair_attention(pr)

    return nc


_NC_CACHE = None


def _get_nc():
    global _NC_CACHE
    if _NC_CACHE is None:
        _NC_CACHE = _build_program()
    return _NC_CACHE


def _prep_inputs(hidden_states, Wq, Wk, Wv):
    x = np.asarray(hidden_states, dtype=np.float32).reshape(S, D)
    xh = np.ascontiguousarray(
        x.reshape(NQC, QC, NDC, P).transpose(3, 0, 2, 1).reshape(P, S * NDC)
    ).astype(np.float16)
    tri = np.triu(np.ones((P, P), dtype=np.float16))

    def wprep(W, h):
        Wh = np.asarray(W, dtype=np.float32)[h * HD:(h + 1) * HD, :]  # [o, in]
        return np.ascontiguousarray(
            Wh.reshape(HD, NDC, P).transpose(2, 1, 0).reshape(P, D)
        ).astype(np.float16)

    in_maps = []
    for h in range(H):
        in_maps.append({
            "xh": xh,
            "wq": wprep(Wq, h),
            "wk": wprep(Wk, h),
            "wv": wprep(Wv, h),
            "tri": tri,
        })
    return in_maps


def kernel(hidden_states, Wq, Wk, Wv, trace=False, **trace_kwargs):
    from concourse.bass_utils import run_bass_kernel_spmd

    in_maps = _prep_inputs(hidden_states, Wq, Wk, Wv)
    nc = _get_nc()
    res = run_bass_kernel_spmd(
        nc, in_maps, core_ids=list(range(H)), trace=trace, **trace_kwargs
    )
    ctx = np.empty((B, S, D), dtype=np.float32)
    for h in range(H):
        ctx[0, :, h * HD:(h + 1) * HD] = res.results[h]["out"].T.astype(np.float32)
    if trace:
        return ctx, res
    return ctx


# revision 16
# speedup vs baseline: 1.0461x; 1.0074x over previous
"""GQA attention kernel for Trainium2: B=1, S=4096, D=1024, H=8 heads (hd=128).

Sharding: one head per NeuronCore (8 cores). Each core computes its head's
Q/K/V projections from the full hidden states, then causal flash-style
attention on-chip, writing its context slice as ctx^T [hd, S] (fp16, host
transposes + upcasts).

Per-core design (fp16 matmul operands, fp32 PSUM):
  - projections and attention are INTERLEAVED: after projecting chunks
    2p,2p+1 of S, the attention for q-chunk pair (2p, 2p+1) is emitted (it
    only needs K/V blocks up to chunk 2p+1), so ACT exp / DVE accum backlogs
    drain under the next chunks' projection matmuls and the PE never idles
  - V^T -> V-natural via DMA XBAR transpose (off the PE)
  - score pairs land in [128,1024] PSUM tiles so exp is one wide ACT op
  - diagonal tiles (both chunks') are emitted FIRST within a pair so their
    serial exp->mask(GPSIMD)->accum chains hide under later full tiles
  - causal diagonal tiles compute only the live slice; mask = one shared
    [128,128] upper-tri multiply on the triangle
  - softmax denominator: DVE accumulates exp tiles into 2 fp16 accumulators
    per q-chunk; TWO accumulating ones-matmuls replicate Z across partitions
    (no DVE merge); reciprocal_approx_fast
  - PV runs in emission order (sum order is free), trailing by 2 tiles
"""

import os
from contextlib import ExitStack

import numpy as np

B, S, D = 1, 4096, 1024
H = 8
HD = D // H  # 128
P = 128
QC = 512  # q-chunk (columns per scores tile)
NDC = D // P  # 8 d-chunks
NQC = S // QC  # 8 q-chunks
NKB = S // P  # 32 k-blocks
SCALE = 1.0 / float(np.sqrt(HD))
LAG = 2  # PV trails score-tile emission by this many tiles
CHUNK = NDC * QC


def _build_program():
    nc = _build_program_inner()
    nc.finalize()
    return nc


def _build_program_inner():
    from concourse import bacc, mybir, tile

    f32 = mybir.dt.float32
    f16 = mybir.dt.float16

    nc = bacc.Bacc("TRN2", target_bir_lowering=False, debug=False)

    # xh[p, n*4096 + d*512 + c] = x[512n + c, 128d + p]
    xh = nc.dram_tensor("xh", [P, S * NDC], f16, kind="ExternalInput")
    # w*[p, d*128 + o] = W[128h + o, 128d + p] for this core's head h
    wq = nc.dram_tensor("wq", [P, D], f16, kind="ExternalInput")
    wk = nc.dram_tensor("wk", [P, D], f16, kind="ExternalInput")
    wv = nc.dram_tensor("wv", [P, D], f16, kind="ExternalInput")
    # tri[r, c] = 1.0 if c >= r else 0.0 (upper triangular incl. diagonal)
    tri = nc.dram_tensor("tri", [P, P], f16, kind="ExternalInput")
    out = nc.dram_tensor("out", [HD, S], f16, kind="ExternalOutput")

    Exp = mybir.ActivationFunctionType.Exp

    with ExitStack() as stack:
        tc = stack.enter_context(tile.TileContext(nc))
        constp = stack.enter_context(tc.tile_pool(name="const", bufs=1))
        qkvp = stack.enter_context(tc.tile_pool(name="qkv", bufs=1))
        xp = stack.enter_context(tc.tile_pool(name="x", bufs=1))
        wp = stack.enter_context(tc.tile_pool(name="w", bufs=1))
        vtp = stack.enter_context(tc.tile_pool(name="vt", bufs=1))
        expp = stack.enter_context(tc.tile_pool(name="expp", bufs=10))
        expb = stack.enter_context(tc.tile_pool(name="expb", bufs=8))
        accp = stack.enter_context(tc.tile_pool(name="accp", bufs=8))
        finp = stack.enter_context(tc.tile_pool(name="fin", bufs=2))
        csp = stack.enter_context(tc.tile_pool(name="csp", bufs=2))
        pp1 = stack.enter_context(tc.tile_pool(name="pp1", bufs=2, space="PSUM"))
        ps_s = stack.enter_context(tc.tile_pool(name="ps_s", bufs=2, space="PSUM"))
        ps_c = stack.enter_context(tc.tile_pool(name="ps_c", bufs=2, space="PSUM"))

        ones_sq = constp.tile([P, P], f16, tag="ones_sq")
        nc.gpsimd.memset(ones_sq[:], 1.0)
        tri_sb = constp.tile([P, P], f16, tag="tri")

        qt_sb = qkvp.tile([P, S], f16, tag="qt")
        kt_sb = qkvp.tile([P, S], f16, tag="kt")
        vn_sb = qkvp.tile([P, S], f16, tag="vn")  # V natural: 32 blocks [128k,128hd]
        xsb = xp.tile([P, S * NDC], f16, tag="xsb")
        vt_sb = vtp.tile([P, S], f16, tag="vt")

        w_sb = {}
        for name in ("q", "k", "v"):
            w_sb[name] = wp.tile([P, D], f16, tag=f"w{name}", name=f"w{name}")
        # Queue split: sync carries only small latency-critical transfers
        # (weights, tri, V transposes, output); scalar carries the bulk xh
        # prefetch so it never delays them. Chunks 0-1 stream per-d-slice so
        # the first projection matmul starts after ~130KB, not 1MB.
        nc.sync.dma_start(out=w_sb["k"][:], in_=wk[:, :])
        for n in (0, 1):
            for d in range(NDC):
                lo = n * CHUNK + d * QC
                nc.scalar.dma_start(out=xsb[:, lo:lo + QC], in_=xh[:, lo:lo + QC])
        nc.sync.dma_start(out=w_sb["q"][:], in_=wq[:, :])
        nc.sync.dma_start(out=w_sb["v"][:], in_=wv[:, :])
        nc.sync.dma_start(out=tri_sb[:], in_=tri[:, :])
        for n in range(2, NQC):
            nc.scalar.dma_start(
                out=xsb[:, n * CHUNK:(n + 1) * CHUNK],
                in_=xh[:, n * CHUNK:(n + 1) * CHUNK],
            )

        def proj_chunk(n):
            xbase = n * CHUNK
            for name, dst in (("k", kt_sb), ("q", qt_sb), ("v", vt_sb)):
                ps = pp1.tile([P, QC], f32, tag="qkvps", name=f"ps{n}{name}")
                for d in range(NDC):
                    nc.tensor.matmul(
                        out=ps[:],
                        lhsT=w_sb[name][:, d * HD:(d + 1) * HD],
                        rhs=xsb[:, xbase + d * QC:xbase + (d + 1) * QC],
                        start=(d == 0),
                        stop=(d == NDC - 1),
                    )
                nc.vector.tensor_copy(out=dst[:, n * QC:(n + 1) * QC], in_=ps[:])

        def pair_attention(pr):
            qA, qB = 2 * pr, 2 * pr + 1
            nkA, nkB = 4 * qA + 4, 4 * qB + 4
            baseA, baseB = qA * QC, qB * QC

            accA = [
                accp.tile([P, QC], f16, tag="acc", name=f"accA{pr}{i}")
                for i in range(2)
            ]
            accB = [
                accp.tile([P, QC], f16, tag="acc", name=f"accB{pr}{i}")
                for i in range(2)
            ]
            c_psA = ps_c.tile([P, QC], f32, tag="cps", name=f"cA{pr}")
            c_psB = ps_c.tile([P, QC], f32, tag="cps", name=f"cB{pr}")

            # emission order: B diagonals, A diagonals (as pair tiles), then
            # full-full pair tiles in k order. PV consumes pair tiles in
            # emission order, then the B diagonals LAST — their vn blocks come
            # from the freshest DMA transpose, so consuming them late gives
            # the transpose time to land without stalling the PE.
            b_diags = list(range(nkA, nkB))
            pair_tiles = list(range(4 * qA, nkA)) + list(range(4 * qA))
            order = b_diags + pair_tiles
            order_pv = pair_tiles + b_diags
            firstA = firstB = pair_tiles[0]
            lastA = pair_tiles[-1]
            lastB = b_diags[-1]
            # per-qc accumulator bookkeeping: emission counts per chunk
            na_seen = [0]
            nb_seen = [0]
            exps = {}  # ki -> (e_tile, loA or None, loB)

            def acc_update(acc, seen, lo, e_ap):
                i = seen[0]
                seen[0] += 1
                par = i % 2
                if i == 0:
                    nc.vector.tensor_copy(out=acc[0][:], in_=e_ap)
                elif i == 1:
                    if lo > 0:
                        nc.vector.memset(acc[1][:, :lo], 0.0)
                    nc.vector.tensor_copy(out=acc[1][:, lo:], in_=e_ap)
                else:
                    nc.vector.tensor_add(
                        out=acc[par][:, lo:], in0=acc[par][:, lo:], in1=e_ap
                    )

            def pv(k):
                e, loA, loB = exps[k]
                if loA is not None:
                    nc.tensor.matmul(
                        out=c_psA[:, loA:],
                        lhsT=vn_sb[:, k * P:(k + 1) * P],
                        rhs=e[:, loA:QC],
                        start=(k == firstA),
                        stop=(k == lastA),
                    )
                nc.tensor.matmul(
                    out=c_psB[:, loB:],
                    lhsT=vn_sb[:, k * P:(k + 1) * P],
                    rhs=e[:, QC + loB:2 * QC] if loA is not None
                    else e[:, loB:QC],
                    start=(k == firstB),
                    stop=(k == lastB),
                )

            def qc_tail(q0, acc, c_ps, base):
                z_ps = ps_s.tile([P, 2 * QC], f32, tag="sps", name=f"z{q0}")
                nc.tensor.matmul(
                    out=z_ps[:, :QC], lhsT=ones_sq[:], rhs=acc[0][:],
                    start=True, stop=False,
                )
                nc.tensor.matmul(
                    out=z_ps[:, :QC], lhsT=ones_sq[:], rhs=acc[1][:],
                    start=False, stop=True,
                )
                rz = finp.tile([P, QC], f32, tag="rz", name=f"rz{q0}")
                nc.vector.reciprocal_approx_fast(out=rz[:], in_=z_ps[:, :QC])
                cs = csp.tile([P, QC], f16, tag="cs", name=f"cs{q0}")
                nc.vector.tensor_mul(out=cs[:], in0=c_ps[:], in1=rz[:])
                nc.sync.dma_start(out=out[:, base:base + QC], in_=cs[:])

            npv = 0  # PVs issued so far (indexes order_pv)
            for idx, ki in enumerate(order):
                loB = P * (ki - 4 * qB) if ki >= 4 * qB else 0
                if ki < nkA:  # pair tile (A live, B full)
                    loA = P * (ki - 4 * qA) if ki >= 4 * qA else 0
                    s = ps_s.tile([P, 2 * QC], f32, tag="sps", name=f"s{pr}_{ki}")
                    nc.tensor.matmul(
                        out=s[:, loA:QC],
                        lhsT=kt_sb[:, ki * P:(ki + 1) * P],
                        rhs=qt_sb[:, baseA + loA:baseA + QC],
                        start=True, stop=True,
                    )
                    nc.tensor.matmul(
                        out=s[:, QC:],
                        lhsT=kt_sb[:, ki * P:(ki + 1) * P],
                        rhs=qt_sb[:, baseB:baseB + QC],
                        start=True, stop=True,
                    )
                    e = expp.tile([P, 2 * QC], f16, tag="exp", name=f"e{pr}_{ki}")
                    nc.scalar.activation(
                        out=e[:, loA:], in_=s[:, loA:], func=Exp, scale=SCALE
                    )
                    if ki >= 4 * qA:  # diagonal for A: mask the triangle
                        nc.gpsimd.tensor_mul(
                            out=e[:, loA:loA + P], in0=e[:, loA:loA + P],
                            in1=tri_sb[:],
                        )
                    acc_update(accA, na_seen, loA, e[:, loA:QC])
                    acc_update(accB, nb_seen, 0, e[:, QC:])
                    exps[ki] = (e, loA, 0)
                else:  # B-only diagonal tile, emitted first
                    s = ps_s.tile([P, 2 * QC], f32, tag="sps", name=f"s{pr}_{ki}")
                    nc.tensor.matmul(
                        out=s[:, loB:QC],
                        lhsT=kt_sb[:, ki * P:(ki + 1) * P],
                        rhs=qt_sb[:, baseB + loB:baseB + QC],
                        start=True, stop=True,
                    )
                    e = expb.tile([P, 2 * QC], f16, tag="expb", name=f"e{pr}_{ki}")
                    nc.scalar.activation(
                        out=e[:, loB:QC], in_=s[:, loB:QC], func=Exp, scale=SCALE
                    )
                    nc.gpsimd.tensor_mul(
                        out=e[:, loB:loB + P], in0=e[:, loB:loB + P],
                        in1=tri_sb[:],
                    )
                    acc_update(accB, nb_seen, loB, e[:, loB:QC])
                    exps[ki] = (e, None, loB)
                # PV trails pair-tile emission by LAG tiles (B-diag emissions
                # don't advance the PV stream; their PVs run at the end)
                j = idx - len(b_diags)  # pair-tile emission index
                if j >= LAG:
                    pv(order_pv[npv])
                    npv += 1
            while npv < len(order_pv):
                pv(order_pv[npv])
                npv += 1
            qc_tail(qA, accA, c_psA, baseA)
            qc_tail(qB, accB, c_psB, baseB)

        # ------------- interleaved projection + attention schedule -------------
        def vtrans(n):
            # V^T chunk n -> V natural blocks via DMA XBAR transpose; issued
            # per chunk so the transfer lands before the pair's PVs need it
            lo, hi = n * QC, (n + 1) * QC
            nc.sync.dma_start_transpose(
                out=vn_sb[:, lo:hi].rearrange("p (b c) -> p b c", c=P),
                in_=vt_sb[:, lo:hi],
            )

        for pr in range(NQC // 2):
            proj_chunk(2 * pr)
            vtrans(2 * pr)
            proj_chunk(2 * pr + 1)
            vtrans(2 * pr + 1)
            pair_attention(pr)

    return nc


_NC_CACHE = None


def _get_nc():
    global _NC_CACHE
    if _NC_CACHE is None:
        _NC_CACHE = _build_program()
    return _NC_CACHE


def _prep_inputs(hidden_states, Wq, Wk, Wv):
    x = np.asarray(hidden_states, dtype=np.float32).reshape(S, D)
    xh = np.ascontiguousarray(
        x.reshape(NQC, QC, NDC, P).transpose(3, 0, 2, 1).reshape(P, S * NDC)
    ).astype(np.float16)
    tri = np.triu(np.ones((P, P), dtype=np.float16))

    def wprep(W, h):
        Wh = np.asarray(W, dtype=np.float32)[h * HD:(h + 1) * HD, :]  # [o, in]
        return np.ascontiguousarray(
            Wh.reshape(HD, NDC, P).transpose(2, 1, 0).reshape(P, D)
        ).astype(np.float16)

    in_maps = []
    for h in range(H):
        in_maps.append({
            "xh": xh,
            "wq": wprep(Wq, h),
            "wk": wprep(Wk, h),
            "wv": wprep(Wv, h),
            "tri": tri,
        })
    return in_maps


def kernel(hidden_states, Wq, Wk, Wv, trace=False, **trace_kwargs):
    from concourse.bass_utils import run_bass_kernel_spmd

    in_maps = _prep_inputs(hidden_states, Wq, Wk, Wv)
    nc = _get_nc()
    res = run_bass_kernel_spmd(
        nc, in_maps, core_ids=list(range(H)), trace=trace, **trace_kwargs
    )
    ctx = np.empty((B, S, D), dtype=np.float32)
    for h in range(H):
        ctx[0, :, h * HD:(h + 1) * HD] = res.results[h]["out"].T.astype(np.float32)
    if trace:
        return ctx, res
    return ctx
